# revision 9
# baseline (speedup 1.0000x reference)
"""Trainium2 Bass kernel for per-skill actor-critic MoE routing.

Strategy (expert-parallel): sort tokens by skill on the host, pad each
skill's token group to capacity C, assign 2 skills per NeuronCore
(8 cores x 2 = 16 skills). Each core runs both skills' tokens through
their actor/critic MLPs in a feature-major layout (features on SBUF
partitions, tokens on the free dim) so no transposes are needed:

  L1: psum[128, n] = [W1; b1].T @ [x.T; 1]   (K=65 contraction, fp32r)
  h1 = tanh(psum)                            (ScalarE, bias folded in L1)
  L2: psum = W2.T @ h1                       (K=128)
  h2 = tanh(psum + b2)                       (ScalarE, bias via ACT bias AP)
  L3: logits.T = W3a.T @ h2a ; value = W3c.T @ h2c   (written to PSUM,
      DMA'd straight to DRAM; output bias added on the host)

The host then scatters per-skill results back to original token order.
"""

import numpy as np

B, D, S, H, A = 32768, 64, 16, 128, 17
NCORES = 8
TILE = 512
SPAN = 1024  # tokens per pipeline iteration (2 matmul tiles)

_compiled = {}


def _build(C):
    """Build + compile the SPMD Tile kernel for per-skill capacity C."""
    import concourse.bass as bass  # noqa: F401
    import concourse.mybir as mybir
    import concourse.tile as tile
    from concourse import bacc

    f32 = mybir.dt.float32
    f32r = mybir.dt.float32r
    Tanh = mybir.ActivationFunctionType.Tanh

    nc = bacc.Bacc("TRN2", target_bir_lowering=False, debug=False,
                   num_devices=NCORES)

    xin = nc.dram_tensor("xin", [D + 1, 2 * C], f32r, kind="ExternalInput")
    l1w = nc.dram_tensor("l1w", [D + 1, 4 * H], f32r, kind="ExternalInput")
    l2w = nc.dram_tensor("l2w", [H, 4 * H], f32r, kind="ExternalInput")
    l2b = nc.dram_tensor("l2b", [H, 4], f32, kind="ExternalInput")
    l3w = nc.dram_tensor("l3w", [H, 4 * (A + 1)], f32r, kind="ExternalInput")
    out = nc.dram_tensor("out", [A + 1, 2 * C], f32, kind="ExternalOutput")

    spans = []
    off = 0
    while off < C:
        w = min(SPAN, C - off)
        spans.append((off, w))
        off += w

    with tile.TileContext(nc) as tc:
        with (
            tc.tile_pool(name="w", bufs=1) as wpool,
            tc.tile_pool(name="x", bufs=3) as xpool,
            tc.tile_pool(name="h", bufs=2) as hpool,
            tc.tile_pool(name="o", bufs=2) as opool,
            tc.tile_pool(name="ps", bufs=1, space="PSUM") as pspool,
        ):
            l1w_sb = wpool.tile([D + 1, 4 * H], f32r)
            nc.sync.dma_start(l1w_sb[:], l1w[:])
            l2w_sb = wpool.tile([H, 4 * H], f32r)
            nc.sync.dma_start(l2w_sb[:], l2w[:])
            l2b_sb = wpool.tile([H, 4], f32)
            nc.sync.dma_start(l2b_sb[:], l2b[:])
            l3w_sb = wpool.tile([H, 4 * (A + 1)], f32r)
            nc.sync.dma_start(l3w_sb[:], l3w[:])

            for s in range(2):
                for off, W in spans:
                    k = W // TILE
                    col0 = s * C + off

                    xt = xpool.tile([D + 1, SPAN], f32r, tag="x")
                    nc.sync.dma_start(xt[:, :W], xin[:, col0:col0 + W])

                    # L1 (actor cols [0:W], critic cols [W:2W]); bias rides
                    # on the ones row of xt (K = D+1 = 65).
                    l1ps = pspool.tile([H, 2 * SPAN], f32, tag="l1")
                    for net in range(2):
                        lw = l1w_sb[:, (2 * s + net) * H:(2 * s + net + 1) * H]
                        for j in range(k):
                            c = (net * k + j) * TILE
                            nc.tensor.matmul(
                                l1ps[:, c:c + TILE],
                                lw,
                                xt[:, j * TILE:(j + 1) * TILE],
                            )
                    h1 = hpool.tile([H, 2 * SPAN], f32r, tag="h1")
                    nc.scalar.activation(h1[:, :2 * W], l1ps[:, :2 * W], Tanh)

                    # L2
                    l2ps = pspool.tile([H, 2 * SPAN], f32, tag="l2")
                    for net in range(2):
                        lw = l2w_sb[:, (2 * s + net) * H:(2 * s + net + 1) * H]
                        for j in range(k):
                            c = (net * k + j) * TILE
                            nc.tensor.matmul(
                                l2ps[:, c:c + TILE],
                                lw,
                                h1[:, c:c + TILE],
                            )
                    h2 = hpool.tile([H, 2 * SPAN], f32r, tag="h2")
                    nc.scalar.activation(h2[:, :W], l2ps[:, :W], Tanh,
                                         bias=l2b_sb[:, 2 * s:2 * s + 1])
                    nc.scalar.activation(h2[:, W:2 * W], l2ps[:, W:2 * W], Tanh,
                                         bias=l2b_sb[:, 2 * s + 1:2 * s + 2])

                    # L3: reuse the l2 psum tile (already consumed by ACT).
                    # Two accumulating matmuls stack logits and value into
                    # one [A+1, n] psum region: [Wa3|0].T @ h2a then
                    # += [0|Wc3].T @ h2c. Output bias added host-side.
                    wa3 = l3w_sb[:, 2 * s * (A + 1):(2 * s + 1) * (A + 1)]
                    wc3 = l3w_sb[:, (2 * s + 1) * (A + 1):(2 * s + 2) * (A + 1)]
                    for j in range(k):
                        c = j * TILE
                        nc.tensor.matmul(
                            l2ps[0:A + 1, c:c + TILE],
                            wa3,
                            h2[:, c:c + TILE],
                            start=True, stop=False,
                        )
                        nc.tensor.matmul(
                            l2ps[0:A + 1, c:c + TILE],
                            wc3,
                            h2[:, W + c:W + c + TILE],
                            start=False, stop=True,
                        )
                    osb = opool.tile([A + 1, SPAN], f32, tag="o")
                    nc.vector.tensor_copy(osb[:, :W], l2ps[0:A + 1, 0:W])
                    nc.sync.dma_start(out[:, col0:col0 + W], osb[:, :W])

    nc.compile()
    return nc


def _get_kernel(C):
    if C not in _compiled:
        _compiled[C] = _build(C)
    return _compiled[C]


def kernel(obs, skill_ids, Wa1, ba1, Wa2, ba2, Wa3, ba3,
           Wc1, bc1, Wc2, bc2, Wc3, bc3):
    from concourse.bass_utils import run_bass_kernel_spmd

    obs = np.asarray(obs, dtype=np.float32)
    sids = np.asarray(skill_ids).astype(np.int64)
    Wa1, ba1, Wa2, ba2, Wa3, ba3 = [np.asarray(a, np.float32)
                                    for a in (Wa1, ba1, Wa2, ba2, Wa3, ba3)]
    Wc1, bc1, Wc2, bc2, Wc3, bc3 = [np.asarray(a, np.float32)
                                    for a in (Wc1, bc1, Wc2, bc2, Wc3, bc3)]

    counts = np.bincount(sids, minlength=S)
    order = np.argsort(sids, kind="stable")
    starts = np.zeros(S + 1, np.int64)
    starts[1:] = np.cumsum(counts)

    C = max(2560, int(-(-counts.max() // TILE) * TILE))
    nc = _get_kernel(C)

    obsT = np.ascontiguousarray(obs.T)  # [D, B]

    in_maps = []
    for c in range(NCORES):
        xin = np.zeros((D + 1, 2 * C), np.float32)
        l1w = np.zeros((D + 1, 4 * H), np.float32)
        l2w = np.zeros((H, 4 * H), np.float32)
        l2b = np.zeros((H, 4), np.float32)
        l3w = np.zeros((H, 4 * (A + 1)), np.float32)
        for sloc in range(2):
            skill = 2 * c + sloc
            cnt = int(counts[skill])
            toks = order[starts[skill]:starts[skill] + cnt]
            xin[:D, sloc * C:sloc * C + cnt] = obsT[:, toks]
            xin[D, sloc * C:sloc * C + cnt] = 1.0
            for net, (W1, b1, W2, b2) in enumerate(
                ((Wa1, ba1, Wa2, ba2), (Wc1, bc1, Wc2, bc2))
            ):
                blk = slice((2 * sloc + net) * H, (2 * sloc + net + 1) * H)
                l1w[:D, blk] = W1[skill]
                l1w[D, blk] = b1[skill]
                l2w[:, blk] = W2[skill]
                l2b[:, 2 * sloc + net] = b2[skill]
            l3w[:, 2 * sloc * (A + 1):2 * sloc * (A + 1) + A] = Wa3[skill]
            l3w[:, (2 * sloc + 1) * (A + 1) + A] = Wc3[skill, :, 0]
        in_maps.append({"xin": xin, "l1w": l1w, "l2w": l2w,
                        "l2b": l2b, "l3w": l3w})

    res = run_bass_kernel_spmd(nc, in_maps, core_ids=list(range(NCORES)))

    logits = np.empty((B, A), np.float32)
    value = np.empty((B,), np.float32)
    for c in range(NCORES):
        o = res.results[c]["out"]
        for sloc in range(2):
            skill = 2 * c + sloc
            cnt = int(counts[skill])
            if cnt == 0:
                continue
            toks = order[starts[skill]:starts[skill] + cnt]
            logits[toks] = o[0:A, sloc * C:sloc * C + cnt].T + ba3[skill]
            value[toks] = o[A, sloc * C:sloc * C + cnt] + bc3[skill, 0]
    return logits, value


# revision 11
# speedup vs baseline: 1.2555x; 1.2555x over previous
"""Trainium2 Bass kernel for per-skill actor-critic MoE routing.

Strategy (expert-parallel): sort tokens by skill on the host, pad each
skill's token group to capacity C, assign 2 skills per NeuronCore
(8 cores x 2 = 16 skills). Each core runs both skills' tokens through
their actor/critic MLPs in a feature-major layout (features on SBUF
partitions, tokens on the free dim) so no transposes are needed:

  L1: psum[128, n] = [W1; b1].T @ [x.T; 1]   (K=65 contraction, fp32r)
  h1 = tanh(psum)                            (ScalarE, bias folded in L1)
  L2: psum = W2.T @ h1                       (K=128)
  h2 = tanh(psum + b2)                       (ScalarE, bias via ACT bias AP)
  L3: per 512-token tile j of the span, accumulate
        [..zeros.. Wa3|0 ..zeros..].T @ h2a_j   (rows 18j..18j+16)
      + [..zeros.. 0|Wc3 ..zeros..].T @ h2c_j   (row  18j+17)
      into one [36, 512] psum block, so a single narrow copy + DMA
      extracts a whole span's logits+value. Output bias added on host.

The host then scatters per-skill results back to original token order.
"""

import numpy as np

B, D, S, H, A = 32768, 64, 16, 128, 17
NCORES = 8
TILE = 512
SPAN = 1024  # tokens per pipeline iteration (2 matmul tiles)
G = A + 1    # output rows per tile group (17 logits + 1 value)

_compiled = {}


def _build(C):
    """Build + compile the SPMD Tile kernel for per-skill capacity C."""
    import concourse.mybir as mybir
    import concourse.tile as tile
    from concourse import bacc

    f32 = mybir.dt.float32
    f32r = mybir.dt.float32r
    Tanh = mybir.ActivationFunctionType.Tanh

    nc = bacc.Bacc("TRN2", target_bir_lowering=False, debug=False,
                   num_devices=NCORES)

    KT = SPAN // TILE  # tiles per full span
    M3 = KT * G        # stacked L3 output rows (36)

    xin = nc.dram_tensor("xin", [D + 1, 2 * C], f32r, kind="ExternalInput")
    l1w = nc.dram_tensor("l1w", [D + 1, 4 * H], f32r, kind="ExternalInput")
    l2w = nc.dram_tensor("l2w", [H, 4 * H], f32r, kind="ExternalInput")
    l2b = nc.dram_tensor("l2b", [H, 4], f32, kind="ExternalInput")
    # 2 skills x 2 nets x KT tile-slots, each a [H, M3] block
    l3w = nc.dram_tensor("l3w", [H, 2 * 2 * KT * M3], f32r,
                         kind="ExternalInput")
    # out[g, tile, :] rows: g in [0,36): tile-in-span output rows
    out = nc.dram_tensor("out", [M3, (2 * C) // TILE // KT, TILE], f32,
                         kind="ExternalOutput")

    assert C % SPAN == 0, "C must be a multiple of SPAN"
    spans = [(o, SPAN) for o in range(0, C, SPAN)]

    with tile.TileContext(nc) as tc:
        with (
            tc.tile_pool(name="w", bufs=1) as wpool,
            tc.tile_pool(name="x", bufs=4) as xpool,
            tc.tile_pool(name="h", bufs=2) as hpool,
            tc.tile_pool(name="o", bufs=3) as opool,
            tc.tile_pool(name="ps", bufs=1, space="PSUM") as pspool,
        ):
            l1w_sb = wpool.tile([D + 1, 4 * H], f32r)
            nc.gpsimd.dma_start(l1w_sb[:], l1w[:])
            l2w_sb = wpool.tile([H, 4 * H], f32r)
            nc.gpsimd.dma_start(l2w_sb[:], l2w[:])
            l2b_sb = wpool.tile([H, 4], f32)
            nc.gpsimd.dma_start(l2b_sb[:], l2b[:])
            l3w_sb = wpool.tile([H, 2 * 2 * KT * M3], f32r)
            nc.gpsimd.dma_start(l3w_sb[:], l3w[:])

            for si, s in enumerate(range(2)):
                for sp, (off, W) in enumerate(spans):
                    k = W // TILE
                    col0 = s * C + off
                    span_idx = (s * C + off) // SPAN

                    xt = xpool.tile([D + 1, SPAN], f32r, tag="x")
                    nc.sync.dma_start(xt[:, :W], xin[:, col0:col0 + W])

                    # L1 (actor cols [0:W], critic cols [W:2W])
                    l1ps = pspool.tile([H, 2 * SPAN], f32, tag="l1")
                    for net in range(2):
                        lw = l1w_sb[:, (2 * s + net) * H:(2 * s + net + 1) * H]
                        for j in range(k):
                            c = (net * k + j) * TILE
                            nc.tensor.matmul(
                                l1ps[:, c:c + TILE], lw,
                                xt[:, j * TILE:(j + 1) * TILE])
                    h1 = hpool.tile([H, 2 * SPAN], f32r, tag="h1")
                    nc.scalar.activation(h1[:, :2 * W], l1ps[:, :2 * W], Tanh)

                    # L2: separate psum per net so h2a ACT can start early
                    l2psa = pspool.tile([H, SPAN], f32, tag="l2a")
                    l2psc = pspool.tile([H, SPAN], f32, tag="l2c")
                    h2 = hpool.tile([H, 2 * SPAN], f32r, tag="h2")
                    for net, ps in ((0, l2psa), (1, l2psc)):
                        lw = l2w_sb[:, (2 * s + net) * H:(2 * s + net + 1) * H]
                        for j in range(k):
                            nc.tensor.matmul(
                                ps[:, j * TILE:(j + 1) * TILE], lw,
                                h1[:, (net * k + j) * TILE:
                                   (net * k + j + 1) * TILE])
                        nc.scalar.activation(
                            h2[:, net * W:net * W + W], ps[:, :W], Tanh,
                            bias=l2b_sb[:, 2 * s + net:2 * s + net + 1])

                    # L3: accumulate the span's logits+value into a stacked
                    # [M3, TILE] block of the (consumed) l2psa tile.
                    for j in range(k):
                        for net in range(2):
                            blk = ((s * 2 + net) * KT + j) * M3
                            nc.tensor.matmul(
                                l2psa[0:M3, 0:TILE],
                                l3w_sb[:, blk:blk + M3],
                                h2[:, (net * k + j) * TILE:
                                   (net * k + j + 1) * TILE],
                                start=(j == 0 and net == 0),
                                stop=(j == k - 1 and net == 1),
                            )
                    osb = opool.tile([M3, TILE], f32, tag="o")
                    nc.vector.tensor_copy(osb[:], l2psa[0:M3, 0:TILE])
                    nc.sync.dma_start(out[:, span_idx, :], osb[:])

    nc.compile()
    return nc


def _get_kernel(C):
    if C not in _compiled:
        _compiled[C] = _build(C)
    return _compiled[C]


def kernel(obs, skill_ids, Wa1, ba1, Wa2, ba2, Wa3, ba3,
           Wc1, bc1, Wc2, bc2, Wc3, bc3):
    from concourse.bass_utils import run_bass_kernel_spmd

    obs = np.asarray(obs, dtype=np.float32)
    sids = np.asarray(skill_ids).astype(np.int64)
    Wa1, ba1, Wa2, ba2, Wa3, ba3 = [np.asarray(a, np.float32)
                                    for a in (Wa1, ba1, Wa2, ba2, Wa3, ba3)]
    Wc1, bc1, Wc2, bc2, Wc3, bc3 = [np.asarray(a, np.float32)
                                    for a in (Wc1, bc1, Wc2, bc2, Wc3, bc3)]

    counts = np.bincount(sids, minlength=S)
    order = np.argsort(sids, kind="stable")
    starts = np.zeros(S + 1, np.int64)
    starts[1:] = np.cumsum(counts)

    C = max(2560, int(-(-counts.max() // SPAN) * SPAN))
    nc = _get_kernel(C)

    KT = SPAN // TILE
    M3 = KT * G

    obsT = np.ascontiguousarray(obs.T)  # [D, B]

    in_maps = []
    for c in range(NCORES):
        xin = np.zeros((D + 1, 2 * C), np.float32)
        l1w = np.zeros((D + 1, 4 * H), np.float32)
        l2w = np.zeros((H, 4 * H), np.float32)
        l2b = np.zeros((H, 4), np.float32)
        l3w = np.zeros((H, 2 * 2 * KT * M3), np.float32)
        for sloc in range(2):
            skill = 2 * c + sloc
            cnt = int(counts[skill])
            toks = order[starts[skill]:starts[skill] + cnt]
            xin[:D, sloc * C:sloc * C + cnt] = obsT[:, toks]
            xin[D, sloc * C:sloc * C + cnt] = 1.0
            for net, (W1, b1, W2, b2) in enumerate(
                ((Wa1, ba1, Wa2, ba2), (Wc1, bc1, Wc2, bc2))
            ):
                blk = slice((2 * sloc + net) * H, (2 * sloc + net + 1) * H)
                l1w[:D, blk] = W1[skill]
                l1w[D, blk] = b1[skill]
                l2w[:, blk] = W2[skill]
                l2b[:, 2 * sloc + net] = b2[skill]
            for j in range(KT):
                ab = ((sloc * 2 + 0) * KT + j) * M3 + j * G
                cb = ((sloc * 2 + 1) * KT + j) * M3 + j * G
                l3w[:, ab:ab + A] = Wa3[skill]
                l3w[:, cb + A] = Wc3[skill, :, 0]
        in_maps.append({"xin": xin, "l1w": l1w, "l2w": l2w,
                        "l2b": l2b, "l3w": l3w})

    res = run_bass_kernel_spmd(nc, in_maps, core_ids=list(range(NCORES)))

    logits = np.empty((B, A), np.float32)
    value = np.empty((B,), np.float32)
    for c in range(NCORES):
        o = res.results[c]["out"]  # [M3, nspans, TILE]
        # token t (in core-column space) = sp*SPAN + j*TILE + u lives at
        # o[j*G + g, sp, u]
        per = o.reshape(KT, G, -1, TILE)  # [KT, G, nspans, TILE]
        per = per.transpose(2, 0, 3, 1)   # [nspans, KT, TILE, G]
        per = per.reshape(-1, G)          # [2C, G] token-major
        for sloc in range(2):
            skill = 2 * c + sloc
            cnt = int(counts[skill])
            if cnt == 0:
                continue
            toks = order[starts[skill]:starts[skill] + cnt]
            blk = per[sloc * C:sloc * C + cnt]
            logits[toks] = blk[:, 0:A] + ba3[skill]
            value[toks] = blk[:, A] + bc3[skill, 0]
    return logits, value


# revision 15
# speedup vs baseline: 1.3934x; 1.1099x over previous
"""Trainium2 Bass kernel for per-skill actor-critic MoE routing.

Strategy (expert-parallel): sort tokens by skill on the host, pad each
skill's token group to capacity C, assign 2 skills per NeuronCore
(8 cores x 2 = 16 skills). Each core runs both skills' tokens through
their actor/critic MLPs in a feature-major layout (features on SBUF
partitions, tokens on the free dim) so no transposes are needed:

  L1: psum[128, n] = [W1; b1].T @ [x.T; 1]   (K=65 contraction, fp32r)
  h1 = tanh(psum)                            (ScalarE, bias folded in L1)
  L2: psum = W2.T @ h1                       (K=128)
  h2 = tanh(psum + b2)                       (ScalarE, bias via ACT bias AP)
  L3: per 512-token tile j of the span, accumulate
        [..zeros.. Wa3|0 ..zeros..].T @ h2a_j   (rows 18j..18j+16)
      + [..zeros.. 0|Wc3 ..zeros..].T @ h2c_j   (row  18j+17)
      into one [36, 512] psum block, so a single narrow copy + DMA
      extracts a whole span's logits+value. Output bias added on host.

The host then scatters per-skill results back to original token order.
"""

import numpy as np

B, D, S, H, A = 32768, 64, 16, 128, 17
NCORES = 8
TILE = 512
SPAN = 1024  # tokens per pipeline iteration (2 matmul tiles)
G = A + 1    # output rows per tile group (17 logits + 1 value)

_compiled = {}


def _build(C):
    """Build + compile the SPMD Tile kernel for per-skill capacity C."""
    import concourse.mybir as mybir
    import concourse.tile as tile
    from concourse import bacc

    f32 = mybir.dt.float32
    f32r = mybir.dt.float32r
    Tanh = mybir.ActivationFunctionType.Tanh

    nc = bacc.Bacc("TRN2", target_bir_lowering=False, debug=False,
                   num_devices=NCORES)

    KT = SPAN // TILE  # tiles per full span
    M3 = KT * G        # stacked L3 output rows (36)

    xin = nc.dram_tensor("xin", [D + 1, 2 * C], f32r, kind="ExternalInput")
    l1w = nc.dram_tensor("l1w", [D + 1, 4 * H], f32r, kind="ExternalInput")
    l2w = nc.dram_tensor("l2w", [H, 4 * H], f32r, kind="ExternalInput")
    l2b = nc.dram_tensor("l2b", [H, 4], f32, kind="ExternalInput")
    # 2 skills x 2 nets x KT tile-slots, each a [H, M3] block
    l3w = nc.dram_tensor("l3w", [H, 2 * 2 * KT * M3], f32r,
                         kind="ExternalInput")

    spans = []
    off = 0
    while off < C:
        w = min(SPAN, C - off)
        spans.append((off, w))
        off += w
    nspans = 2 * len(spans)
    out = nc.dram_tensor("out", [M3, nspans, TILE], f32,
                         kind="ExternalOutput")

    with tile.TileContext(nc) as tc:
        with (
            tc.tile_pool(name="w", bufs=1) as wpool,
            tc.tile_pool(name="x", bufs=4) as xpool,
            tc.tile_pool(name="h", bufs=2) as hpool,
            tc.tile_pool(name="o", bufs=3) as opool,
            tc.tile_pool(name="ps", bufs=1, space="PSUM") as pspool,
        ):
            l1w_sb = wpool.tile([D + 1, 4 * H], f32r)
            nc.gpsimd.dma_start(l1w_sb[:], l1w[:])
            l2w_sb = wpool.tile([H, 4 * H], f32r)
            nc.gpsimd.dma_start(l2w_sb[:], l2w[:])
            l2b_sb = wpool.tile([H, 4], f32)
            nc.gpsimd.dma_start(l2b_sb[:], l2b[:])
            l3w_sb = wpool.tile([H, 2 * 2 * KT * M3], f32r)
            nc.gpsimd.dma_start(l3w_sb[:], l3w[:])

            for s in range(2):
                for sp, (off, W) in enumerate(spans):
                    k = W // TILE
                    col0 = s * C + off
                    span_idx = s * len(spans) + sp

                    xt = xpool.tile([D + 1, SPAN], f32r, tag="x")
                    nc.sync.dma_start(xt[:, :W], xin[:, col0:col0 + W])

                    # L1 (actor cols [0:W], critic cols [W:2W])
                    l1ps = pspool.tile([H, 2 * SPAN], f32, tag="l1")
                    for net in range(2):
                        lw = l1w_sb[:, (2 * s + net) * H:(2 * s + net + 1) * H]
                        for j in range(k):
                            c = (net * k + j) * TILE
                            nc.tensor.matmul(
                                l1ps[:, c:c + TILE], lw,
                                xt[:, j * TILE:(j + 1) * TILE])
                    h1 = hpool.tile([H, 2 * SPAN], f32r, tag="h1")
                    nc.scalar.activation(h1[:, :2 * W], l1ps[:, :2 * W], Tanh)

                    # L2: separate psum per net so h2a ACT can start early
                    l2psa = pspool.tile([H, SPAN], f32, tag="l2a")
                    l2psc = pspool.tile([H, SPAN], f32, tag="l2c")
                    h2 = hpool.tile([H, 2 * SPAN], f32r, tag="h2")
                    for net, ps in ((0, l2psa), (1, l2psc)):
                        lw = l2w_sb[:, (2 * s + net) * H:(2 * s + net + 1) * H]
                        for j in range(k):
                            nc.tensor.matmul(
                                ps[:, j * TILE:(j + 1) * TILE], lw,
                                h1[:, (net * k + j) * TILE:
                                   (net * k + j + 1) * TILE])
                        nc.scalar.activation(
                            h2[:, net * W:net * W + W], ps[:, :W], Tanh,
                            bias=l2b_sb[:, 2 * s + net:2 * s + net + 1])

                    # L3: accumulate the span's logits+value into a stacked
                    # [M3, TILE] block of the (consumed) l2psa tile.
                    for j in range(k):
                        for net in range(2):
                            blk = ((s * 2 + net) * KT + j) * M3
                            nc.tensor.matmul(
                                l2psa[0:M3, 0:TILE],
                                l3w_sb[:, blk:blk + M3],
                                h2[:, (net * k + j) * TILE:
                                   (net * k + j + 1) * TILE],
                                start=(j == 0 and net == 0),
                                stop=(j == k - 1 and net == 1),
                            )
                    osb = opool.tile([M3, TILE], f32, tag="o")
                    nc.vector.tensor_copy(osb[:], l2psa[0:M3, 0:TILE])
                    nc.sync.dma_start(out[:, span_idx, :], osb[:])

    nc.compile()
    return nc


def _get_kernel(C):
    if C not in _compiled:
        _compiled[C] = _build(C)
    return _compiled[C]


def kernel(obs, skill_ids, Wa1, ba1, Wa2, ba2, Wa3, ba3,
           Wc1, bc1, Wc2, bc2, Wc3, bc3):
    from concourse.bass_utils import run_bass_kernel_spmd

    obs = np.asarray(obs, dtype=np.float32)
    sids = np.asarray(skill_ids).astype(np.int64)
    Wa1, ba1, Wa2, ba2, Wa3, ba3 = [np.asarray(a, np.float32)
                                    for a in (Wa1, ba1, Wa2, ba2, Wa3, ba3)]
    Wc1, bc1, Wc2, bc2, Wc3, bc3 = [np.asarray(a, np.float32)
                                    for a in (Wc1, bc1, Wc2, bc2, Wc3, bc3)]

    counts = np.bincount(sids, minlength=S)
    order = np.argsort(sids, kind="stable")
    starts = np.zeros(S + 1, np.int64)
    starts[1:] = np.cumsum(counts)

    C = max(2560, int(-(-counts.max() // TILE) * TILE))
    nc = _get_kernel(C)

    KT = SPAN // TILE
    M3 = KT * G
    spans = []
    o = 0
    while o < C:
        w = min(SPAN, C - o)
        spans.append((o, w))
        o += w

    obsT = np.ascontiguousarray(obs.T)  # [D, B]

    in_maps = []
    for c in range(NCORES):
        xin = np.zeros((D + 1, 2 * C), np.float32)
        l1w = np.zeros((D + 1, 4 * H), np.float32)
        l2w = np.zeros((H, 4 * H), np.float32)
        l2b = np.zeros((H, 4), np.float32)
        l3w = np.zeros((H, 2 * 2 * KT * M3), np.float32)
        for sloc in range(2):
            skill = 2 * c + sloc
            cnt = int(counts[skill])
            toks = order[starts[skill]:starts[skill] + cnt]
            xin[:D, sloc * C:sloc * C + cnt] = obsT[:, toks]
            xin[D, sloc * C:sloc * C + cnt] = 1.0
            for net, (W1, b1, W2, b2) in enumerate(
                ((Wa1, ba1, Wa2, ba2), (Wc1, bc1, Wc2, bc2))
            ):
                blk = slice((2 * sloc + net) * H, (2 * sloc + net + 1) * H)
                l1w[:D, blk] = W1[skill]
                l1w[D, blk] = b1[skill]
                l2w[:, blk] = W2[skill]
                l2b[:, 2 * sloc + net] = b2[skill]
            for j in range(KT):
                ab = ((sloc * 2 + 0) * KT + j) * M3 + j * G
                cb = ((sloc * 2 + 1) * KT + j) * M3 + j * G
                l3w[:, ab:ab + A] = Wa3[skill]
                l3w[:, cb + A] = Wc3[skill, :, 0]
        in_maps.append({"xin": xin, "l1w": l1w, "l2w": l2w,
                        "l2b": l2b, "l3w": l3w})

    res = run_bass_kernel_spmd(nc, in_maps, core_ids=list(range(NCORES)))

    logits = np.empty((B, A), np.float32)
    value = np.empty((B,), np.float32)
    for c in range(NCORES):
        ot = res.results[c]["out"]  # [M3, nspans, TILE]
        # token (off + j*TILE + u) of span slot sp lives at ot[j*G + g, sp, u]
        per = np.empty((2 * C, G), np.float32)
        for sloc in range(2):
            for sp, (off, W) in enumerate(spans):
                slot = sloc * len(spans) + sp
                for j in range(W // TILE):
                    blk = ot[j * G:(j + 1) * G, slot, :]  # [G, TILE]
                    t0 = sloc * C + off + j * TILE
                    per[t0:t0 + TILE] = blk.T
        for sloc in range(2):
            skill = 2 * c + sloc
            cnt = int(counts[skill])
            if cnt == 0:
                continue
            toks = order[starts[skill]:starts[skill] + cnt]
            blk = per[sloc * C:sloc * C + cnt]
            logits[toks] = blk[:, 0:A] + ba3[skill]
            value[toks] = blk[:, A] + bc3[skill, 0]
    return logits, value


# revision 16
# speedup vs baseline: 1.4126x; 1.0138x over previous
"""Trainium2 Bass kernel for per-skill actor-critic MoE routing.

Expert-parallel: tokens are sorted by skill on the host, each skill's
group padded to capacity C, two skills per NeuronCore. Each core runs
its tokens through the per-skill actor/critic MLPs in a feature-major
layout (features on SBUF partitions, tokens on the free dim):

  L1: psum = [W1; b1].T/a @ [x.T; 1]      (K=65, fp32r, bias+1/a folded)
  h1 = tanh(a * psum)                     (ScalarE tanh, or a 2-pass
       custom-DVE polynomial composite with max err ~2e-3, to split the
       activation load across both engines)
  L2: psum = W2.T @ h1                    (K=128, fp32r)
  h2 = tanh(psum + b2)                    (ScalarE, bias via ACT bias AP)
  L3: per 512-token tile j, two accumulating matmuls with zero-padded
      [H, 36] weight blocks stack logits+value of up to 2 tiles into one
      [36, 512] psum block -> one narrow copy + DMA per span.

Host adds the L3 output bias and scatters back to original token order.
"""

import numpy as np

B, D, S, H, A = 32768, 64, 16, 128, 17
NCORES = 8
TILE = 512
SPAN = 1024
G = A + 1
KT = SPAN // TILE
M3 = KT * G

# composite-tanh params (max abs err 2.0e-3 vs tanh, inputs prescaled 1/a)
A_SCALE = 3.7599417433556486
B0 = 2.8540707547137547
B1 = -3.003233876981482
B2 = 1.438174591155331
E0 = 1.304379694345473
E1 = -0.5021730968337124
E2 = 0.11673136204346712

_compiled = {}
_tanh_ops = None


def _register_tanh_ops():
    """Register the 2-pass composite-tanh custom DVE ops (idempotent).

    pass1: w = uc*(C3 + y*(C2 + y*C1)), uc = clamp(Src0, C0, 1), y = uc^2
    pass2: v = clamp(Src0*(C0 + z*(C2 + z*C1)), C3, 1), z = Src0^2
    (C3 rides in1 as a [P,1] broadcast; exactly 8 ALU slices each.)
    """
    global _tanh_ops
    if _tanh_ops is not None:
        return _tanh_ops

    from concourse import dve_ops
    from concourse.dve_spec import (
        Spec, Src0, C0, C1, C2, C3, One, minn, maxx, lower,
    )
    from concourse.dve_ops import DveOp, _spill_c3_to_src1
    from concourse.dve_uop import DveOpSpec

    def _p1_ref(in0, in1, s0, s1, imm2):
        u = np.minimum(in0.astype(np.float32), 1.0)
        u = np.maximum(u, s0)
        y = u * u
        return u * (in1 + y * (imm2 + y * s1))

    def _p2_ref(in0, in1, s0, s1, imm2):
        w = in0.astype(np.float32)
        z = w * w
        v = w * (s0 + z * (imm2 + z * s1))
        return np.maximum(np.minimum(v, 1.0), in1)

    _uc = maxx(minn(Src0, One), C0)
    _y = _uc * _uc
    p1_body = _uc * (C3 + _y * (C2 + _y * C1))
    _z = Src0 * Src0
    p2_body = maxx(minn(Src0 * (C0 + _z * (C2 + _z * C1)), One), C3)

    ops = []
    for name, body, ref in (
        ("TANH_P1_ANT", p1_body, _p1_ref),
        ("TANH_P2_ANT", p2_body, _p2_ref),
    ):
        if name in dve_ops.CUSTOM_DVE_SPECS:
            ops.append(next(o for o in dve_ops.OPS if o.name == name))
            continue
        spec = Spec(body=_spill_c3_to_src1(body), reference=ref)
        opcode = dve_ops._CUSTOM_DVE_ROW_BASE + len(dve_ops.OPS)
        shas = {}
        for ver in ("v3", "v4"):
            try:
                sp = DveOpSpec(name=name, opcode=opcode,
                               uops=lower(spec, ver=ver), rd1_en=True)
                shas[ver] = sp.sha(ver)
            except Exception:
                pass
        op = DveOp(name, spec, subdim=False, uops_sha=shas)
        dve_ops.OPS.append(op)
        dve_ops.CUSTOM_DVE_SPECS[name] = spec
        dve_ops._SUB_OPCODE_FOR_NAME[name] = opcode
        ops.append(op)
    _tanh_ops = tuple(ops)
    return _tanh_ops


def _spans_for(C):
    spans = []
    off = 0
    while off < C:
        w = min(SPAN, C - off)
        spans.append((off, w))
        off += w
    return spans


def _tiles_for(W):
    tiles = []
    o = 0
    while o < W:
        w = min(TILE, W - o)
        tiles.append((o, w))
        o += w
    return tiles


def _build(C):
    """Build + compile the SPMD Tile kernel for per-skill capacity C."""
    import concourse.mybir as mybir
    import concourse.tile as tile
    from concourse import bacc

    P1, P2 = _register_tanh_ops()

    f32 = mybir.dt.float32
    f32r = mybir.dt.float32r
    Tanh = mybir.ActivationFunctionType.Tanh

    nc = bacc.Bacc("TRN2", target_bir_lowering=False, debug=False,
                   num_devices=NCORES)

    xin = nc.dram_tensor("xin", [D + 1, 2 * C], f32r, kind="ExternalInput")
    l1w = nc.dram_tensor("l1w", [D + 1, 4 * H], f32r, kind="ExternalInput")
    l2w = nc.dram_tensor("l2w", [H, 4 * H], f32r, kind="ExternalInput")
    l2b = nc.dram_tensor("l2b", [H, 4], f32, kind="ExternalInput")
    l3w = nc.dram_tensor("l3w", [H, 2 * 2 * KT * M3], f32r,
                         kind="ExternalInput")

    spans = _spans_for(C)
    nspans = 2 * len(spans)
    out = nc.dram_tensor("out", [M3, nspans, TILE], f32,
                         kind="ExternalOutput")

    # h1 of these (skill, span) pairs runs on the DVE composite; the rest
    # (and all h2) use exact ScalarE tanh.
    dve_h1 = {(0, 0), (1, 0)}

    with tile.TileContext(nc) as tc:
        with (
            tc.tile_pool(name="w", bufs=1) as wpool,
            tc.tile_pool(name="x", bufs=4) as xpool,
            tc.tile_pool(name="h", bufs=2) as hpool,
            tc.tile_pool(name="o", bufs=3) as opool,
            tc.tile_pool(name="ps", bufs=1, space="PSUM") as pspool,
        ):
            # L1 weights first on the fast queue: first matmuls need them
            l1w_sb = wpool.tile([D + 1, 4 * H], f32r)
            nc.sync.dma_start(l1w_sb[:], l1w[:])
            l2w_sb = wpool.tile([H, 4 * H], f32r)
            nc.gpsimd.dma_start(l2w_sb[:], l2w[:])
            l2b_sb = wpool.tile([H, 4], f32)
            nc.gpsimd.dma_start(l2b_sb[:], l2b[:])
            l3w_sb = wpool.tile([H, 2 * 2 * KT * M3], f32r)
            nc.gpsimd.dma_start(l3w_sb[:], l3w[:])
            # [P,1] constants for the custom ops' spilled C3 operand
            cconst = wpool.tile([H, 2], f32)
            nc.gpsimd.memset(cconst[:, 0:1], B0)
            nc.gpsimd.memset(cconst[:, 1:2], -1.0)

            for s in range(2):
                for sp, (off, W) in enumerate(spans):
                    tiles = _tiles_for(W)
                    col0 = s * C + off
                    slot = s * len(spans) + sp

                    xt = xpool.tile([D + 1, SPAN], f32r, tag="x")
                    nc.sync.dma_start(xt[:, :W], xin[:, col0:col0 + W])

                    # L1 (actor cols [0:W], critic cols [W:2W])
                    l1ps = pspool.tile([H, 2 * SPAN], f32, tag="l1")
                    for net in range(2):
                        lw = l1w_sb[:, (2 * s + net) * H:(2 * s + net + 1) * H]
                        for (to, tw) in tiles:
                            c = net * W + to
                            nc.tensor.matmul(l1ps[:, c:c + tw], lw,
                                             xt[:, to:to + tw])
                    h1 = hpool.tile([H, 2 * SPAN], f32r, tag="h1")
                    if (s, sp) in dve_h1:
                        wt = hpool.tile([H, 2 * SPAN], f32, tag="wt")
                        nc.vector._custom_dve(
                            P1, out=wt[:, :2 * W], in0=l1ps[:, :2 * W],
                            in1=cconst[:, 0:1], s0=-1.0, s1=B2, imm2=B1)
                        nc.vector._custom_dve(
                            P2, out=h1[:, :2 * W], in0=wt[:, :2 * W],
                            in1=cconst[:, 1:2], s0=E0, s1=E2, imm2=E1)
                    else:
                        nc.scalar.activation(h1[:, :2 * W], l1ps[:, :2 * W],
                                             Tanh, scale=float(A_SCALE))

                    # L2: separate psum per net so h2a ACT starts early
                    l2psa = pspool.tile([H, SPAN], f32, tag="l2a")
                    l2psc = pspool.tile([H, SPAN], f32, tag="l2c")
                    h2 = hpool.tile([H, 2 * SPAN], f32r, tag="h2")
                    for net, ps in ((0, l2psa), (1, l2psc)):
                        lw = l2w_sb[:, (2 * s + net) * H:(2 * s + net + 1) * H]
                        for (to, tw) in tiles:
                            nc.tensor.matmul(
                                ps[:, to:to + tw], lw,
                                h1[:, net * W + to:net * W + to + tw])
                        nc.scalar.activation(
                            h2[:, net * W:net * W + W], ps[:, :W], Tanh,
                            bias=l2b_sb[:, 2 * s + net:2 * s + net + 1])

                    # L3: stack the span's logits+value into [M3, tw] of the
                    # (consumed) l2psa tile via accumulating matmuls.
                    n_mm = 2 * len(tiles)
                    mi = 0
                    for j, (to, tw) in enumerate(tiles):
                        for net in range(2):
                            blk = ((s * 2 + net) * KT + j) * M3
                            nc.tensor.matmul(
                                l2psa[0:M3, 0:tw],
                                l3w_sb[:, blk:blk + M3],
                                h2[:, net * W + to:net * W + to + tw],
                                start=(mi == 0), stop=(mi == n_mm - 1),
                            )
                            mi += 1
                    osb = opool.tile([M3, TILE], f32, tag="o")
                    tw0 = tiles[0][1]
                    nc.vector.tensor_copy(osb[:, :tw0], l2psa[0:M3, 0:tw0])
                    nc.sync.dma_start(out[:, slot, :tw0], osb[:, :tw0])

    nc.compile()
    return nc


def _get_kernel(C):
    if C not in _compiled:
        _compiled[C] = _build(C)
    return _compiled[C]


def kernel(obs, skill_ids, Wa1, ba1, Wa2, ba2, Wa3, ba3,
           Wc1, bc1, Wc2, bc2, Wc3, bc3):
    from concourse.bass_utils import run_bass_kernel_spmd

    obs = np.asarray(obs, dtype=np.float32)
    sids = np.asarray(skill_ids).astype(np.int64)
    Wa1, ba1, Wa2, ba2, Wa3, ba3 = [np.asarray(a, np.float32)
                                    for a in (Wa1, ba1, Wa2, ba2, Wa3, ba3)]
    Wc1, bc1, Wc2, bc2, Wc3, bc3 = [np.asarray(a, np.float32)
                                    for a in (Wc1, bc1, Wc2, bc2, Wc3, bc3)]

    counts = np.bincount(sids, minlength=S)
    order = np.argsort(sids, kind="stable")
    starts = np.zeros(S + 1, np.int64)
    starts[1:] = np.cumsum(counts)

    C = max(2304, int(-(-counts.max() // 256) * 256))
    nc = _get_kernel(C)
    spans = _spans_for(C)

    inv_a = np.float32(1.0 / A_SCALE)
    obsT = np.ascontiguousarray(obs.T)  # [D, B]

    in_maps = []
    for c in range(NCORES):
        xin = np.zeros((D + 1, 2 * C), np.float32)
        l1w = np.zeros((D + 1, 4 * H), np.float32)
        l2w = np.zeros((H, 4 * H), np.float32)
        l2b = np.zeros((H, 4), np.float32)
        l3w = np.zeros((H, 2 * 2 * KT * M3), np.float32)
        for sloc in range(2):
            skill = 2 * c + sloc
            cnt = int(counts[skill])
            toks = order[starts[skill]:starts[skill] + cnt]
            xin[:D, sloc * C:sloc * C + cnt] = obsT[:, toks]
            xin[D, sloc * C:sloc * C + cnt] = 1.0
            for net, (W1, b1, W2, b2) in enumerate(
                ((Wa1, ba1, Wa2, ba2), (Wc1, bc1, Wc2, bc2))
            ):
                blk = slice((2 * sloc + net) * H, (2 * sloc + net + 1) * H)
                l1w[:D, blk] = W1[skill] * inv_a
                l1w[D, blk] = b1[skill] * inv_a
                l2w[:, blk] = W2[skill]
                l2b[:, 2 * sloc + net] = b2[skill]
            for j in range(KT):
                ab = ((sloc * 2 + 0) * KT + j) * M3 + j * G
                cb = ((sloc * 2 + 1) * KT + j) * M3 + j * G
                l3w[:, ab:ab + A] = Wa3[skill]
                l3w[:, cb + A] = Wc3[skill, :, 0]
        in_maps.append({"xin": xin, "l1w": l1w, "l2w": l2w,
                        "l2b": l2b, "l3w": l3w})

    res = run_bass_kernel_spmd(nc, in_maps, core_ids=list(range(NCORES)))

    logits = np.empty((B, A), np.float32)
    value = np.empty((B,), np.float32)
    for c in range(NCORES):
        ot = res.results[c]["out"]  # [M3, nspans, TILE]
        per = np.empty((2 * C, G), np.float32)
        for sloc in range(2):
            for sp, (off, W) in enumerate(spans):
                slot = sloc * len(spans) + sp
                for j, (to, tw) in enumerate(_tiles_for(W)):
                    blk = ot[j * G:(j + 1) * G, slot, :tw]  # [G, tw]
                    t0 = sloc * C + off + to
                    per[t0:t0 + tw] = blk.T
        for sloc in range(2):
            skill = 2 * c + sloc
            cnt = int(counts[skill])
            if cnt == 0:
                continue
            toks = order[starts[skill]:starts[skill] + cnt]
            blk = per[sloc * C:sloc * C + cnt]
            logits[toks] = blk[:, 0:A] + ba3[skill]
            value[toks] = blk[:, A] + bc3[skill, 0]
    return logits, value


# revision 18
# speedup vs baseline: 1.4219x; 1.0066x over previous
"""Trainium2 Bass kernel for per-skill actor-critic MoE routing.

Expert-parallel: tokens are sorted by skill on the host, each skill's
group padded to capacity C, two skills per NeuronCore. Each core runs
its tokens through the per-skill actor/critic MLPs in a feature-major
layout (features on SBUF partitions, tokens on the free dim):

  L1: psum = [W1; b1].T/a @ [x.T; 1]      (K=65, fp32r, bias+1/a folded)
  h1 = tanh(a * psum)                     (ScalarE tanh, or a 2-pass
       custom-DVE polynomial composite with max err ~2e-3, to split the
       activation load across both engines)
  L2: psum = W2.T @ h1                    (K=128, fp32r)
  h2 = tanh(psum + b2)                    (ScalarE, bias via ACT bias AP)
  L3: per 512-token tile j, two accumulating matmuls with zero-padded
      [H, 36] weight blocks stack logits+value of up to 2 tiles into one
      [36, 512] psum block -> one narrow copy + DMA per span.

Host adds the L3 output bias and scatters back to original token order.
"""

import numpy as np

B, D, S, H, A = 32768, 64, 16, 128, 17
NCORES = 8
TILE = 512
SPAN = 1024
G = A + 1
KT = SPAN // TILE
M3 = KT * G

# composite-tanh params (max abs err 2.0e-3 vs tanh, inputs prescaled 1/a)
A_SCALE = 3.7599417433556486
B0 = 2.8540707547137547
B1 = -3.003233876981482
B2 = 1.438174591155331
E0 = 1.304379694345473
E1 = -0.5021730968337124
E2 = 0.11673136204346712

_compiled = {}
_tanh_ops = None


def _register_tanh_ops():
    """Register the 2-pass composite-tanh custom DVE ops (idempotent).

    pass1: w = uc*(C3 + y*(C2 + y*C1)), uc = clamp(Src0, C0, 1), y = uc^2
    pass2: v = clamp(Src0*(C0 + z*(C2 + z*C1)), C3, 1), z = Src0^2
    (C3 rides in1 as a [P,1] broadcast; exactly 8 ALU slices each.)
    """
    global _tanh_ops
    if _tanh_ops is not None:
        return _tanh_ops

    from concourse import dve_ops
    from concourse.dve_spec import (
        Spec, Src0, C0, C1, C2, C3, One, minn, maxx, lower,
    )
    from concourse.dve_ops import DveOp, _spill_c3_to_src1
    from concourse.dve_uop import DveOpSpec

    def _p1_ref(in0, in1, s0, s1, imm2):
        u = np.minimum(in0.astype(np.float32), 1.0)
        u = np.maximum(u, s0)
        y = u * u
        return u * (in1 + y * (imm2 + y * s1))

    def _p2_ref(in0, in1, s0, s1, imm2):
        w = in0.astype(np.float32)
        z = w * w
        v = w * (s0 + z * (imm2 + z * s1))
        return np.maximum(np.minimum(v, 1.0), in1)

    _uc = maxx(minn(Src0, One), C0)
    _y = _uc * _uc
    p1_body = _uc * (C3 + _y * (C2 + _y * C1))
    _z = Src0 * Src0
    p2_body = maxx(minn(Src0 * (C0 + _z * (C2 + _z * C1)), One), C3)

    ops = []
    for name, body, ref in (
        ("TANH_P1_ANT", p1_body, _p1_ref),
        ("TANH_P2_ANT", p2_body, _p2_ref),
    ):
        if name in dve_ops.CUSTOM_DVE_SPECS:
            ops.append(next(o for o in dve_ops.OPS if o.name == name))
            continue
        spec = Spec(body=_spill_c3_to_src1(body), reference=ref)
        opcode = dve_ops._CUSTOM_DVE_ROW_BASE + len(dve_ops.OPS)
        shas = {}
        for ver in ("v3", "v4"):
            try:
                sp = DveOpSpec(name=name, opcode=opcode,
                               uops=lower(spec, ver=ver), rd1_en=True)
                shas[ver] = sp.sha(ver)
            except Exception:
                pass
        op = DveOp(name, spec, subdim=False, uops_sha=shas)
        dve_ops.OPS.append(op)
        dve_ops.CUSTOM_DVE_SPECS[name] = spec
        dve_ops._SUB_OPCODE_FOR_NAME[name] = opcode
        ops.append(op)
    _tanh_ops = tuple(ops)
    return _tanh_ops


def _spans_for(C):
    spans = []
    off = 0
    while off < C:
        w = min(SPAN, C - off)
        spans.append((off, w))
        off += w
    return spans


def _tiles_for(W):
    tiles = []
    o = 0
    while o < W:
        w = min(TILE, W - o)
        tiles.append((o, w))
        o += w
    return tiles


def _build(C):
    """Build + compile the SPMD Tile kernel for per-skill capacity C."""
    import concourse.mybir as mybir
    import concourse.tile as tile
    from concourse import bacc

    P1, P2 = _register_tanh_ops()

    f32 = mybir.dt.float32
    f32r = mybir.dt.float32r
    Tanh = mybir.ActivationFunctionType.Tanh

    nc = bacc.Bacc("TRN2", target_bir_lowering=False, debug=False,
                   num_devices=NCORES)

    xin = nc.dram_tensor("xin", [D + 1, 2 * C], f32r, kind="ExternalInput")
    l1w = nc.dram_tensor("l1w", [D + 1, 4 * H], f32r, kind="ExternalInput")
    l2w = nc.dram_tensor("l2w", [H, 4 * H], f32r, kind="ExternalInput")
    l2b = nc.dram_tensor("l2b", [H, 4], f32, kind="ExternalInput")
    l3w = nc.dram_tensor("l3w", [H, 2 * 2 * KT * M3], f32r,
                         kind="ExternalInput")

    spans = _spans_for(C)
    nspans = 2 * len(spans)
    out = nc.dram_tensor("out", [M3, nspans, TILE], f32,
                         kind="ExternalOutput")

    # Unit order: small ragged spans first (fast pipeline fill) and last
    # (fast drain); full spans of the two skills interleaved in between.
    full = [(s, sp) for sp, (o, w) in enumerate(spans) if w == SPAN
            for s in (0, 1)]
    full.sort(key=lambda t: (t[1], t[0]))
    ragged = [(s, sp) for sp, (o, w) in enumerate(spans) if w < SPAN
              for s in (0, 1)]
    units = ragged[:1] + full + ragged[1:]

    # h1 of these (skill, span, net) triples runs on the DVE composite;
    # the rest (and all h2) use exact ScalarE tanh.  Chosen to balance
    # ScalarE vs Vector engine busy time.
    dve_h1 = set()
    for i, (s, sp) in enumerate(full):
        if i % 2 == 0:
            dve_h1 |= {(s, sp, 0), (s, sp, 1)}
        elif i % 4 == 1:
            dve_h1 |= {(s, sp, 0)}

    with tile.TileContext(nc) as tc:
        with (
            tc.tile_pool(name="w", bufs=1) as wpool,
            tc.tile_pool(name="x", bufs=4) as xpool,
            tc.tile_pool(name="h", bufs=3) as hpool,
            tc.tile_pool(name="o", bufs=4) as opool,
            tc.tile_pool(name="ps", bufs=1, space="PSUM") as pspool,
        ):
            # L1 weights first on the fast queue: first matmuls need them
            l1w_sb = wpool.tile([D + 1, 4 * H], f32r)
            nc.sync.dma_start(l1w_sb[:], l1w[:])
            l2w_sb = wpool.tile([H, 4 * H], f32r)
            nc.gpsimd.dma_start(l2w_sb[:], l2w[:])
            l2b_sb = wpool.tile([H, 4], f32)
            nc.gpsimd.dma_start(l2b_sb[:], l2b[:])
            l3w_sb = wpool.tile([H, 2 * 2 * KT * M3], f32r)
            nc.gpsimd.dma_start(l3w_sb[:], l3w[:])
            # [P,1] constants for the custom ops' spilled C3 operand
            cconst = wpool.tile([H, 2], f32)
            nc.gpsimd.memset(cconst[:, 0:1], B0)
            nc.gpsimd.memset(cconst[:, 1:2], -1.0)

            for s, sp in units:
                off, W = spans[sp]
                tiles = _tiles_for(W)
                col0 = s * C + off
                slot = s * len(spans) + sp

                xt = xpool.tile([D + 1, SPAN], f32r, tag="x")
                nc.sync.dma_start(xt[:, :W], xin[:, col0:col0 + W])

                h2s = []
                l2ps_by_net = {}
                for net in range(2):
                    # L1 (double-buffered psum: PE runs ahead of ACT/DVE)
                    l1ps = pspool.tile([H, SPAN], f32, tag="l1", bufs=2)
                    lw = l1w_sb[:, (2 * s + net) * H:(2 * s + net + 1) * H]
                    for (to, tw) in tiles:
                        nc.tensor.matmul(l1ps[:, to:to + tw], lw,
                                         xt[:, to:to + tw])
                    h1 = hpool.tile([H, SPAN], f32r, tag="h1")
                    if (s, sp, net) in dve_h1:
                        wt = hpool.tile([H, SPAN], f32, tag="wt")
                        nc.vector._custom_dve(
                            P1, out=wt[:, :W], in0=l1ps[:, :W],
                            in1=cconst[:, 0:1], s0=-1.0, s1=B2, imm2=B1)
                        nc.vector._custom_dve(
                            P2, out=h1[:, :W], in0=wt[:, :W],
                            in1=cconst[:, 1:2], s0=E0, s1=E2, imm2=E1)
                    else:
                        nc.scalar.activation(h1[:, :W], l1ps[:, :W],
                                             Tanh, scale=float(A_SCALE))

                    # L2
                    l2ps = pspool.tile([H, SPAN], f32,
                                       tag="l2a" if net == 0 else "l2c")
                    l2ps_by_net[net] = l2ps
                    lw = l2w_sb[:, (2 * s + net) * H:(2 * s + net + 1) * H]
                    for (to, tw) in tiles:
                        nc.tensor.matmul(l2ps[:, to:to + tw], lw,
                                         h1[:, to:to + tw])
                    h2 = hpool.tile([H, SPAN], f32r, tag="h2")
                    nc.scalar.activation(
                        h2[:, :W], l2ps[:, :W], Tanh,
                        bias=l2b_sb[:, 2 * s + net:2 * s + net + 1])
                    h2s.append(h2)

                # L3: stack the span's logits+value into [M3, tw] of the
                # (consumed) l2psa tile via accumulating matmuls.
                n_mm = 2 * len(tiles)
                mi = 0
                for j, (to, tw) in enumerate(tiles):
                    for net in range(2):
                        blk = ((s * 2 + net) * KT + j) * M3
                        nc.tensor.matmul(
                            l2ps_by_net[0][0:M3, 0:tw],
                            l3w_sb[:, blk:blk + M3],
                            h2s[net][:, to:to + tw],
                            start=(mi == 0), stop=(mi == n_mm - 1),
                        )
                        mi += 1
                osb = opool.tile([M3, TILE], f32, tag="o")
                tw0 = tiles[0][1]
                nc.vector.tensor_copy(osb[:, :tw0], l2ps_by_net[0][0:M3, 0:tw0])
                nc.sync.dma_start(out[:, slot, :tw0], osb[:, :tw0])

    nc.compile()
    return nc


def _get_kernel(C):
    if C not in _compiled:
        _compiled[C] = _build(C)
    return _compiled[C]


def kernel(obs, skill_ids, Wa1, ba1, Wa2, ba2, Wa3, ba3,
           Wc1, bc1, Wc2, bc2, Wc3, bc3):
    from concourse.bass_utils import run_bass_kernel_spmd

    obs = np.asarray(obs, dtype=np.float32)
    sids = np.asarray(skill_ids).astype(np.int64)
    Wa1, ba1, Wa2, ba2, Wa3, ba3 = [np.asarray(a, np.float32)
                                    for a in (Wa1, ba1, Wa2, ba2, Wa3, ba3)]
    Wc1, bc1, Wc2, bc2, Wc3, bc3 = [np.asarray(a, np.float32)
                                    for a in (Wc1, bc1, Wc2, bc2, Wc3, bc3)]

    counts = np.bincount(sids, minlength=S)
    order = np.argsort(sids, kind="stable")
    starts = np.zeros(S + 1, np.int64)
    starts[1:] = np.cumsum(counts)

    C = max(2304, int(-(-counts.max() // 256) * 256))
    nc = _get_kernel(C)
    spans = _spans_for(C)

    inv_a = np.float32(1.0 / A_SCALE)
    obsT = np.ascontiguousarray(obs.T)  # [D, B]

    in_maps = []
    for c in range(NCORES):
        xin = np.zeros((D + 1, 2 * C), np.float32)
        l1w = np.zeros((D + 1, 4 * H), np.float32)
        l2w = np.zeros((H, 4 * H), np.float32)
        l2b = np.zeros((H, 4), np.float32)
        l3w = np.zeros((H, 2 * 2 * KT * M3), np.float32)
        for sloc in range(2):
            skill = 2 * c + sloc
            cnt = int(counts[skill])
            toks = order[starts[skill]:starts[skill] + cnt]
            xin[:D, sloc * C:sloc * C + cnt] = obsT[:, toks]
            xin[D, sloc * C:sloc * C + cnt] = 1.0
            for net, (W1, b1, W2, b2) in enumerate(
                ((Wa1, ba1, Wa2, ba2), (Wc1, bc1, Wc2, bc2))
            ):
                blk = slice((2 * sloc + net) * H, (2 * sloc + net + 1) * H)
                l1w[:D, blk] = W1[skill] * inv_a
                l1w[D, blk] = b1[skill] * inv_a
                l2w[:, blk] = W2[skill]
                l2b[:, 2 * sloc + net] = b2[skill]
            for j in range(KT):
                ab = ((sloc * 2 + 0) * KT + j) * M3 + j * G
                cb = ((sloc * 2 + 1) * KT + j) * M3 + j * G
                l3w[:, ab:ab + A] = Wa3[skill]
                l3w[:, cb + A] = Wc3[skill, :, 0]
        in_maps.append({"xin": xin, "l1w": l1w, "l2w": l2w,
                        "l2b": l2b, "l3w": l3w})

    res = run_bass_kernel_spmd(nc, in_maps, core_ids=list(range(NCORES)))

    logits = np.empty((B, A), np.float32)
    value = np.empty((B,), np.float32)
    for c in range(NCORES):
        ot = res.results[c]["out"]  # [M3, nspans, TILE]
        per = np.empty((2 * C, G), np.float32)
        for sloc in range(2):
            for sp, (off, W) in enumerate(spans):
                slot = sloc * len(spans) + sp
                for j, (to, tw) in enumerate(_tiles_for(W)):
                    blk = ot[j * G:(j + 1) * G, slot, :tw]  # [G, tw]
                    t0 = sloc * C + off + to
                    per[t0:t0 + tw] = blk.T
        for sloc in range(2):
            skill = 2 * c + sloc
            cnt = int(counts[skill])
            if cnt == 0:
                continue
            toks = order[starts[skill]:starts[skill] + cnt]
            blk = per[sloc * C:sloc * C + cnt]
            logits[toks] = blk[:, 0:A] + ba3[skill]
            value[toks] = blk[:, A] + bc3[skill, 0]
    return logits, value


# revision 38
# speedup vs baseline: 1.5347x; 1.0793x over previous
"""Trainium2 Bass kernel for per-skill actor-critic MoE routing.

Expert-parallel: tokens are sorted by skill on the host, each skill's
group padded to capacity C, two skills per NeuronCore. Each core runs
its tokens through the per-skill actor/critic MLPs in a feature-major
layout (features on SBUF partitions, tokens on the free dim):

  L1: psum = [W1; b1].T/a @ [x.T; 1]      (K=65, fp32r, bias+1/a folded)
  h1 = tanh(a * psum)                     (ScalarE tanh, or a 2-pass
       custom-DVE polynomial composite with max err ~2e-3, to split the
       activation load across both engines)
  L2: psum = W2.T @ h1                    (K=128, fp32r)
  h2 = tanh(psum + b2)                    (ScalarE, bias via ACT bias AP)
  L3: per 512-token tile j, two accumulating matmuls with zero-padded
      [H, 36] weight blocks stack logits+value of up to 2 tiles into one
      [36, 512] psum block -> one narrow copy + DMA per span.

Host adds the L3 output bias and scatters back to original token order.
"""

import numpy as np

B, D, S, H, A = 32768, 64, 16, 128, 17
NCORES = 8
TILE = 512
SPAN = 1024
G = A + 1
KT = SPAN // TILE
M3 = KT * G
M6 = 2 * M3

# composite-tanh params (max abs err 2.0e-3 vs tanh, inputs prescaled 1/a)
A_SCALE = 3.7599417433556486
B0 = 2.8540707547137547
B1 = -3.003233876981482
B2 = 1.438174591155331
E0 = 1.304379694345473
E1 = -0.5021730968337124
E2 = 0.11673136204346712

_compiled = {}
_tanh_ops = None
VARIANT = "C"
DVE_H1_OVERRIDE = None
NWARM = 6
UNIT_ORDER = "rag_both_last"


def _register_tanh_ops():
    """Register the 2-pass composite-tanh custom DVE ops (idempotent).

    pass1: w = uc*(C3 + y*(C2 + y*C1)), uc = clamp(Src0, C0, 1), y = uc^2
    pass2: v = clamp(Src0*(C0 + z*(C2 + z*C1)), C3, 1), z = Src0^2
    (C3 rides in1 as a [P,1] broadcast; exactly 8 ALU slices each.)
    """
    global _tanh_ops
    if _tanh_ops is not None:
        return _tanh_ops

    from concourse import dve_ops
    from concourse.dve_spec import (
        Spec, Src0, C0, C1, C2, C3, One, minn, maxx, lower,
    )
    from concourse.dve_ops import DveOp, _spill_c3_to_src1
    from concourse.dve_uop import DveOpSpec

    def _p1_ref(in0, in1, s0, s1, imm2):
        u = np.minimum(in0.astype(np.float32), 1.0)
        u = np.maximum(u, s0)
        y = u * u
        return u * (in1 + y * (imm2 + y * s1))

    def _p2_ref(in0, in1, s0, s1, imm2):
        w = in0.astype(np.float32)
        z = w * w
        v = w * (s0 + z * (imm2 + z * s1))
        return np.maximum(np.minimum(v, 1.0), in1)

    _uc = maxx(minn(Src0, One), C0)
    _y = _uc * _uc
    p1_body = _uc * (C3 + _y * (C2 + _y * C1))
    _z = Src0 * Src0
    p2_body = maxx(minn(Src0 * (C0 + _z * (C2 + _z * C1)), One), C3)

    ops = []
    for name, body, ref in (
        ("TANH_P1_ANT", p1_body, _p1_ref),
        ("TANH_P2_ANT", p2_body, _p2_ref),
    ):
        if name in dve_ops.CUSTOM_DVE_SPECS:
            ops.append(next(o for o in dve_ops.OPS if o.name == name))
            continue
        spec = Spec(body=_spill_c3_to_src1(body), reference=ref)
        opcode = dve_ops._CUSTOM_DVE_ROW_BASE + len(dve_ops.OPS)
        shas = {}
        for ver in ("v3", "v4"):
            try:
                sp = DveOpSpec(name=name, opcode=opcode,
                               uops=lower(spec, ver=ver), rd1_en=True)
                shas[ver] = sp.sha(ver)
            except Exception:
                pass
        op = DveOp(name, spec, subdim=False, uops_sha=shas)
        dve_ops.OPS.append(op)
        dve_ops.CUSTOM_DVE_SPECS[name] = spec
        dve_ops._SUB_OPCODE_FOR_NAME[name] = opcode
        ops.append(op)
    _tanh_ops = tuple(ops)
    return _tanh_ops


def _spans_for(C):
    spans = []
    off = 0
    while off < C:
        w = min(SPAN, C - off)
        spans.append((off, w))
        off += w
    return spans


def _tiles_for(W):
    tiles = []
    o = 0
    while o < W:
        w = min(TILE, W - o)
        tiles.append((o, w))
        o += w
    return tiles


def _build(C):
    """Build + compile the SPMD Tile kernel for per-skill capacity C."""
    import concourse.mybir as mybir
    import concourse.tile as tile
    from concourse import bacc

    P1, P2 = _register_tanh_ops()

    f32 = mybir.dt.float32
    f32r = mybir.dt.float32r
    Tanh = mybir.ActivationFunctionType.Tanh

    nc = bacc.Bacc("TRN2", target_bir_lowering=False, debug=False,
                   num_devices=NCORES)

    xin = nc.dram_tensor("xin", [D + 1, 2 * C], f32r, kind="ExternalInput")
    l1w = nc.dram_tensor("l1w", [D + 1, 4 * H], f32r, kind="ExternalInput")
    l2w = nc.dram_tensor("l2w", [H, 4 * H], f32r, kind="ExternalInput")
    l2b = nc.dram_tensor("l2b", [H, 4], f32, kind="ExternalInput")
    l3w = nc.dram_tensor("l3w", [H, 2 * 2 * KT * M6], f32r,
                         kind="ExternalInput")

    spans = _spans_for(C)
    out = nc.dram_tensor("out", [M6, len(spans), TILE], f32,
                         kind="ExternalOutput")

    # Unit order: small ragged spans first (fast pipeline fill) and last
    # (fast drain); full spans of the two skills interleaved in between.
    full = [(s, sp) for sp, (o, w) in enumerate(spans) if w == SPAN
            for s in (0, 1)]
    full.sort(key=lambda t: (t[1], t[0]))
    ragged = [(s, sp) for sp, (o, w) in enumerate(spans) if w < SPAN
              for s in (0, 1)]
    if UNIT_ORDER == "rag_first_last":
        units = ragged[:1] + full + ragged[1:]
    elif UNIT_ORDER == "rag_both_last":
        units = full + ragged
    else:
        units = ragged + full

    # h1 of these (skill, span, net) triples runs on the DVE composite;
    # the rest (and all h2) use exact ScalarE tanh.  Chosen to balance
    # ScalarE vs Vector engine busy time (~40% of h1 on DVE).
    if DVE_H1_OVERRIDE is not None:
        dve_h1 = set(DVE_H1_OVERRIDE(full, ragged))
    else:
        dve_h1 = set()
        for i, (s, sp) in enumerate(full):
            dve_h1.add((s, sp, i % 2))
        if ragged:
            s, sp = ragged[0]
            dve_h1 |= {(s, sp, 0), (s, sp, 1)}
        if len(ragged) > 1:
            dve_h1.add((ragged[1][0], ragged[1][1], 0))

    with tile.TileContext(nc) as tc:
        with (
            tc.tile_pool(name="w", bufs=1) as wpool,
            tc.tile_pool(name="x", bufs=6) as xpool,
            tc.tile_pool(name="h", bufs=4) as hpool,
            tc.tile_pool(name="o", bufs=6) as opool,
            tc.tile_pool(name="ps", bufs=1, space="PSUM") as pspool,
        ):
            # L1 weights first (Pool queue, issues immediately); the first
            # x span rides the sync queue in parallel.
            l1w_sb = wpool.tile([D + 1, 4 * H], f32r)
            nc.gpsimd.dma_start(l1w_sb[:], l1w[:])
            l2w_sb = wpool.tile([H, 4 * H], f32r)
            nc.gpsimd.dma_start(l2w_sb[:], l2w[:])
            l2b_sb = wpool.tile([H, 4], f32)
            nc.gpsimd.dma_start(l2b_sb[:], l2b[:])
            l3w_sb = wpool.tile([H, 2 * 2 * KT * M6], f32r)
            nc.gpsimd.dma_start(l3w_sb[:], l3w[:])
            # [P,1] constants for the custom ops' spilled C3 operand
            cconst = wpool.tile([H, 2], f32)
            nc.gpsimd.memset(cconst[:, 0:1], B0)
            nc.gpsimd.memset(cconst[:, 1:2], -1.0)

            # PE warmup during the DMA-bound head: zero matmuls ramp the
            # HAM clock so real matmuls start at full speed.
            zt = wpool.tile([H, TILE], f32)
            nc.vector.memset(zt[:], 0.0)
            # dummy tanh: forces the ACT table load during the DMA head
            ztanh = wpool.tile([H, 1], f32)
            nc.scalar.activation(ztanh[:], cconst[:, 1:2], Tanh)
            for wi in range(NWARM):
                if VARIANT in ("A", "C"):
                    wps = pspool.tile([H, SPAN], f32, tag="l1", bufs=2)
                elif VARIANT == "E":
                    wps = pspool.tile([H, SPAN], f32, tag="l2", bufs=3)
                else:
                    wps = pspool.tile([M3, TILE], f32, tag="l3", bufs=2)
                nc.tensor.matmul(wps[0:16, 0:TILE], zt[:, 0:16], zt[:])

            pair_ps = None
            for s, sp in units:
                off, W = spans[sp]
                tiles = _tiles_for(W)
                col0 = s * C + off
                slot = s * len(spans) + sp

                xt = xpool.tile([D + 1, SPAN], f32r, tag="x")
                nc.sync.dma_start(xt[:, :W], xin[:, col0:col0 + W])

                def do_l1(net):
                    l1ps = pspool.tile(
                        [H, SPAN], f32, tag="l1",
                        bufs=1 if VARIANT in ("B", "E") else 2,
                        name=f"l1ps{net}")
                    lw = l1w_sb[:, (2 * s + net) * H:(2 * s + net + 1) * H]
                    for (to, tw) in tiles:
                        nc.tensor.matmul(l1ps[:, to:to + tw], lw,
                                         xt[:, to:to + tw])
                    return l1ps

                def do_h1(net, l1ps):
                    h1 = hpool.tile([H, SPAN], f32r, tag="h1", name=f"h1_{net}")
                    if (s, sp, net) in dve_h1:
                        wt = hpool.tile([H, SPAN], f32, tag="wt")
                        nc.vector._custom_dve(
                            P1, out=wt[:, :W], in0=l1ps[:, :W],
                            in1=cconst[:, 0:1], s0=-1.0, s1=B2, imm2=B1)
                        nc.vector._custom_dve(
                            P2, out=h1[:, :W], in0=wt[:, :W],
                            in1=cconst[:, 1:2], s0=E0, s1=E2, imm2=E1)
                    else:
                        nc.scalar.activation(h1[:, :W], l1ps[:, :W],
                                             Tanh, scale=float(A_SCALE))
                    return h1

                def do_l2(net, h1):
                    if VARIANT == "E":
                        l2ps = pspool.tile([H, SPAN], f32, tag="l2",
                                           bufs=3, name=f"l2ps{net}")
                    elif VARIANT == "A":
                        l2ps = pspool.tile([H, SPAN], f32, tag="l2",
                                           bufs=2, name=f"l2ps{net}")
                    elif VARIANT == "C":
                        l2ps = []
                        lw = l2w_sb[:, (2 * s + net) * H:
                                    (2 * s + net + 1) * H]
                        for (to, tw) in tiles:
                            ps = pspool.tile([H, TILE], f32, tag="l2",
                                             bufs=3, name=f"l2ps{net}_{to}")
                            nc.tensor.matmul(ps[:, :tw], lw,
                                             h1[:, to:to + tw])
                            l2ps.append(ps)
                        return l2ps
                    else:
                        l2ps = pspool.tile([H, SPAN], f32,
                                           tag="l2a" if net == 0 else "l2c",
                                           name=f"l2ps{net}")
                    lw = l2w_sb[:, (2 * s + net) * H:(2 * s + net + 1) * H]
                    for (to, tw) in tiles:
                        nc.tensor.matmul(l2ps[:, to:to + tw], lw,
                                         h1[:, to:to + tw])
                    return l2ps

                def do_h2(net, l2ps):
                    h2 = hpool.tile([H, SPAN], f32r, tag="h2", name=f"h2_{net}")
                    if VARIANT == "C":
                        for ps, (to, tw) in zip(l2ps, tiles):
                            nc.scalar.activation(
                                h2[:, to:to + tw], ps[:, :tw], Tanh,
                                bias=l2b_sb[:, 2 * s + net:2 * s + net + 1])
                    else:
                        nc.scalar.activation(
                            h2[:, :W], l2ps[:, :W], Tanh,
                            bias=l2b_sb[:, 2 * s + net:2 * s + net + 1])
                    return h2

                h2s = []
                if VARIANT == "C":
                    psa = do_l1(0)
                    psc = do_l1(1)
                    h1a = do_h1(0, psa)
                    h1c = do_h1(1, psc)
                    l2a = do_l2(0, h1a)
                    l2c = do_l2(1, h1c)
                    h2s = [do_h2(0, l2a), do_h2(1, l2c)]
                    if pair_ps is None:
                        pair_ps = pspool.tile([M6, TILE], f32, tag="l3",
                                              bufs=1)
                        pair_first = True
                    else:
                        pair_first = False
                    l3tgt = pair_ps[:]
                elif VARIANT == "E":
                    psa = do_l1(0)
                    psc = do_l1(1)
                    h1a = do_h1(0, psa)
                    h1c = do_h1(1, psc)
                    l2a = do_l2(0, h1a)
                    l2c = do_l2(1, h1c)
                    h2s = [do_h2(0, l2a), do_h2(1, l2c)]
                    l3tgt = l2a[0:M3, 0:TILE]
                else:
                    l2s = []
                    for net in range(2):
                        ps = do_l1(net)
                        h1 = do_h1(net, ps)
                        l2ps = do_l2(net, h1)
                        h2s.append(do_h2(net, l2ps))
                        l2s.append(l2ps)
                    l3full = pspool.tile([M3, TILE], f32, tag="l3", bufs=2)
                    l3tgt = l3full[:]
                n_mm = 2 * len(tiles)
                mi = 0
                for net in range(2):
                    for j, (to, tw) in enumerate(tiles):
                        blk = ((s * 2 + net) * KT + j) * M6
                        nc.tensor.matmul(
                            l3tgt[:, 0:tw],
                            l3w_sb[:, blk:blk + M6],
                            h2s[net][:, to:to + tw],
                            start=(pair_first and mi == 0),
                            stop=(not pair_first and mi == n_mm - 1),
                            skip_group_check=True,
                        )
                        mi += 1
                if not pair_first:
                    osb = opool.tile([M6, TILE], f32, tag="o")
                    tw0 = tiles[0][1]
                    nc.vector.tensor_copy(osb[:, :tw0], l3tgt[:, 0:tw0])
                    nc.sync.dma_start(out[:, sp, :tw0], osb[:, :tw0])
                    pair_ps = None

    nc.compile()
    return nc


def _get_kernel(C):
    if C not in _compiled:
        _compiled[C] = _build(C)
    return _compiled[C]


def kernel(obs, skill_ids, Wa1, ba1, Wa2, ba2, Wa3, ba3,
           Wc1, bc1, Wc2, bc2, Wc3, bc3):
    from concourse.bass_utils import run_bass_kernel_spmd

    obs = np.asarray(obs, dtype=np.float32)
    sids = np.asarray(skill_ids).astype(np.int64)
    Wa1, ba1, Wa2, ba2, Wa3, ba3 = [np.asarray(a, np.float32)
                                    for a in (Wa1, ba1, Wa2, ba2, Wa3, ba3)]
    Wc1, bc1, Wc2, bc2, Wc3, bc3 = [np.asarray(a, np.float32)
                                    for a in (Wc1, bc1, Wc2, bc2, Wc3, bc3)]

    counts = np.bincount(sids, minlength=S)
    order = np.argsort(sids, kind="stable")
    starts = np.zeros(S + 1, np.int64)
    starts[1:] = np.cumsum(counts)

    C = max(2176, int(-(-counts.max() // 128) * 128))
    nc = _get_kernel(C)
    spans = _spans_for(C)

    inv_a = np.float32(1.0 / A_SCALE)
    obsT = np.ascontiguousarray(obs.T)  # [D, B]

    in_maps = []
    for c in range(NCORES):
        xin = np.zeros((D + 1, 2 * C), np.float32)
        l1w = np.zeros((D + 1, 4 * H), np.float32)
        l2w = np.zeros((H, 4 * H), np.float32)
        l2b = np.zeros((H, 4), np.float32)
        l3w = np.zeros((H, 2 * 2 * KT * M6), np.float32)
        for sloc in range(2):
            skill = 2 * c + sloc
            cnt = int(counts[skill])
            toks = order[starts[skill]:starts[skill] + cnt]
            xin[:D, sloc * C:sloc * C + cnt] = obsT[:, toks]
            xin[D, sloc * C:sloc * C + cnt] = 1.0
            for net, (W1, b1, W2, b2) in enumerate(
                ((Wa1, ba1, Wa2, ba2), (Wc1, bc1, Wc2, bc2))
            ):
                blk = slice((2 * sloc + net) * H, (2 * sloc + net + 1) * H)
                l1w[:D, blk] = W1[skill] * inv_a
                l1w[D, blk] = b1[skill] * inv_a
                l2w[:, blk] = W2[skill]
                l2b[:, 2 * sloc + net] = b2[skill]
            for j in range(KT):
                ro = (sloc * KT + j) * G
                ab = ((sloc * 2 + 0) * KT + j) * M6 + ro
                cb = ((sloc * 2 + 1) * KT + j) * M6 + ro
                l3w[:, ab:ab + A] = Wa3[skill]
                l3w[:, cb + A] = Wc3[skill, :, 0]
        in_maps.append({"xin": xin, "l1w": l1w, "l2w": l2w,
                        "l2b": l2b, "l3w": l3w})

    res = run_bass_kernel_spmd(nc, in_maps, core_ids=list(range(NCORES)))

    logits = np.empty((B, A), np.float32)
    value = np.empty((B,), np.float32)
    for c in range(NCORES):
        ot = res.results[c]["out"]  # [M6, len(spans), TILE]
        per = np.empty((2 * C, G), np.float32)
        for sloc in range(2):
            for sp, (off, W) in enumerate(spans):
                for j, (to, tw) in enumerate(_tiles_for(W)):
                    r0 = (sloc * KT + j) * G
                    blk = ot[r0:r0 + G, sp, :tw]  # [G, tw]
                    t0 = sloc * C + off + to
                    per[t0:t0 + tw] = blk.T
        for sloc in range(2):
            skill = 2 * c + sloc
            cnt = int(counts[skill])
            if cnt == 0:
                continue
            toks = order[starts[skill]:starts[skill] + cnt]
            blk = per[sloc * C:sloc * C + cnt]
            logits[toks] = blk[:, 0:A] + ba3[skill]
            value[toks] = blk[:, A] + bc3[skill, 0]
    return logits, value


# revision 39
# speedup vs baseline: 1.9656x; 1.2808x over previous
"""Trainium2 Bass kernel for per-skill actor-critic MoE routing.

Expert-parallel: tokens are sorted by skill on the host, each skill's
group padded to capacity C, two skills per NeuronCore. Each core runs
its tokens through the per-skill actor/critic MLPs in a feature-major
layout (features on SBUF partitions, tokens on the free dim):

  L1: psum = [W1; b1].T/a @ [x.T; 1]      (K=65, fp32r, bias+1/a folded)
  h1 = tanh(a * psum)                     (ScalarE tanh, or a 2-pass
       custom-DVE polynomial composite with max err ~2e-3, to split the
       activation load across both engines)
  L2: psum = W2.T @ h1                    (K=128, fp32r)
  h2 = tanh(psum + b2)                    (ScalarE, bias via ACT bias AP)
  L3: per 512-token tile j, two accumulating matmuls with zero-padded
      [H, 36] weight blocks stack logits+value of up to 2 tiles into one
      [36, 512] psum block -> one narrow copy + DMA per span.

Host adds the L3 output bias and scatters back to original token order.
"""

import numpy as np

B, D, S, H, A = 32768, 64, 16, 128, 17
NCORES = 8
TILE = 512
SPAN = 1024
G = A + 1
KT = SPAN // TILE
M3 = KT * G
M6 = 2 * M3

# composite-tanh params (max abs err 2.0e-3 vs tanh, inputs prescaled 1/a)
A_SCALE = 3.7599417433556486
B0 = 2.8540707547137547
B1 = -3.003233876981482
B2 = 1.438174591155331
E0 = 1.304379694345473
E1 = -0.5021730968337124
E2 = 0.11673136204346712

_compiled = {}
_tanh_ops = None
VARIANT = "C"
DVE_H1_OVERRIDE = None
NWARM = 6
UNIT_ORDER = "rag_both_last"


def _register_tanh_ops():
    """Register the 2-pass composite-tanh custom DVE ops (idempotent).

    pass1: w = uc*(C3 + y*(C2 + y*C1)), uc = clamp(Src0, C0, 1), y = uc^2
    pass2: v = clamp(Src0*(C0 + z*(C2 + z*C1)), C3, 1), z = Src0^2
    (C3 rides in1 as a [P,1] broadcast; exactly 8 ALU slices each.)
    """
    global _tanh_ops
    if _tanh_ops is not None:
        return _tanh_ops

    from concourse import dve_ops
    from concourse.dve_spec import (
        Spec, Src0, C0, C1, C2, C3, One, minn, maxx, lower,
    )
    from concourse.dve_ops import DveOp, _spill_c3_to_src1
    from concourse.dve_uop import DveOpSpec

    def _p1_ref(in0, in1, s0, s1, imm2):
        u = np.minimum(in0.astype(np.float32), 1.0)
        u = np.maximum(u, s0)
        y = u * u
        return u * (in1 + y * (imm2 + y * s1))

    def _p2_ref(in0, in1, s0, s1, imm2):
        w = in0.astype(np.float32)
        z = w * w
        v = w * (s0 + z * (imm2 + z * s1))
        return np.maximum(np.minimum(v, 1.0), in1)

    _uc = maxx(minn(Src0, One), C0)
    _y = _uc * _uc
    p1_body = _uc * (C3 + _y * (C2 + _y * C1))
    _z = Src0 * Src0
    p2_body = maxx(minn(Src0 * (C0 + _z * (C2 + _z * C1)), One), C3)

    ops = []
    for name, body, ref in (
        ("TANH_P1_ANT", p1_body, _p1_ref),
        ("TANH_P2_ANT", p2_body, _p2_ref),
    ):
        if name in dve_ops.CUSTOM_DVE_SPECS:
            ops.append(next(o for o in dve_ops.OPS if o.name == name))
            continue
        spec = Spec(body=_spill_c3_to_src1(body), reference=ref)
        opcode = dve_ops._CUSTOM_DVE_ROW_BASE + len(dve_ops.OPS)
        shas = {}
        for ver in ("v3", "v4"):
            try:
                sp = DveOpSpec(name=name, opcode=opcode,
                               uops=lower(spec, ver=ver), rd1_en=True)
                shas[ver] = sp.sha(ver)
            except Exception:
                pass
        op = DveOp(name, spec, subdim=False, uops_sha=shas)
        dve_ops.OPS.append(op)
        dve_ops.CUSTOM_DVE_SPECS[name] = spec
        dve_ops._SUB_OPCODE_FOR_NAME[name] = opcode
        ops.append(op)
    _tanh_ops = tuple(ops)
    return _tanh_ops


def _spans_for(C):
    spans = []
    off = 0
    while off < C:
        w = min(SPAN, C - off)
        spans.append((off, w))
        off += w
    return spans


def _tiles_for(W):
    tiles = []
    o = 0
    while o < W:
        w = min(TILE, W - o)
        tiles.append((o, w))
        o += w
    return tiles


def _build(C):
    """Build + compile the SPMD Tile kernel for per-skill capacity C."""
    import concourse.mybir as mybir
    import concourse.tile as tile
    from concourse import bacc

    P1, P2 = _register_tanh_ops()

    f32 = mybir.dt.float32
    f32r = mybir.dt.float32r
    Tanh = mybir.ActivationFunctionType.Tanh

    nc = bacc.Bacc("TRN2", target_bir_lowering=False, debug=False,
                   num_devices=NCORES)

    xin = nc.dram_tensor("xin", [D + 1, 2 * C], f32r, kind="ExternalInput")
    l1w = nc.dram_tensor("l1w", [D + 1, 4 * H], f32r, kind="ExternalInput")
    l2w = nc.dram_tensor("l2w", [H, 4 * H], f32r, kind="ExternalInput")
    l2b = nc.dram_tensor("l2b", [H, 4], f32, kind="ExternalInput")
    l3w = nc.dram_tensor("l3w", [H, 2 * 2 * KT * M6], f32r,
                         kind="ExternalInput")

    spans = _spans_for(C)
    out = nc.dram_tensor("out", [M6, len(spans), TILE], f32,
                         kind="ExternalOutput")

    # Unit order: small ragged spans first (fast pipeline fill) and last
    # (fast drain); full spans of the two skills interleaved in between.
    full = [(s, sp) for sp, (o, w) in enumerate(spans) if w == SPAN
            for s in (0, 1)]
    full.sort(key=lambda t: (t[1], t[0]))
    ragged = [(s, sp) for sp, (o, w) in enumerate(spans) if w < SPAN
              for s in (0, 1)]
    if UNIT_ORDER == "rag_first_last":
        units = ragged[:1] + full + ragged[1:]
    elif UNIT_ORDER == "rag_both_last":
        units = full + ragged
    else:
        units = ragged + full

    # h1 of these (skill, span, net) triples runs on the DVE composite;
    # the rest (and all h2) use exact ScalarE tanh.  Chosen to balance
    # ScalarE vs Vector engine busy time (~40% of h1 on DVE).
    if DVE_H1_OVERRIDE is not None:
        dve_h1 = set(DVE_H1_OVERRIDE(full, ragged))
    else:
        dve_h1 = set()
        for i, (s, sp) in enumerate(full):
            dve_h1.add((s, sp, i % 2))
        if ragged:
            s, sp = ragged[0]
            dve_h1 |= {(s, sp, 0), (s, sp, 1)}
        if len(ragged) > 1:
            dve_h1.add((ragged[1][0], ragged[1][1], 0))

    with tile.TileContext(nc) as tc:
        with (
            tc.tile_pool(name="w", bufs=1) as wpool,
            tc.tile_pool(name="x", bufs=6) as xpool,
            tc.tile_pool(name="h", bufs=4) as hpool,
            tc.tile_pool(name="o", bufs=6) as opool,
            tc.tile_pool(name="ps", bufs=1, space="PSUM") as pspool,
        ):
            # L1 weights first (Pool queue, issues immediately); the first
            # x span rides the sync queue in parallel.
            l1w_sb = wpool.tile([D + 1, 4 * H], f32r)
            nc.gpsimd.dma_start(l1w_sb[:], l1w[:])
            l2w_sb = wpool.tile([H, 4 * H], f32r)
            nc.gpsimd.dma_start(l2w_sb[:], l2w[:])
            l2b_sb = wpool.tile([H, 4], f32)
            nc.gpsimd.dma_start(l2b_sb[:], l2b[:])
            l3w_sb = wpool.tile([H, 2 * 2 * KT * M6], f32r)
            nc.gpsimd.dma_start(l3w_sb[:], l3w[:])
            # [P,1] constants for the custom ops' spilled C3 operand
            cconst = wpool.tile([H, 2], f32)
            nc.gpsimd.memset(cconst[:, 0:1], B0)
            nc.gpsimd.memset(cconst[:, 1:2], -1.0)

            # PE warmup during the DMA-bound head: zero matmuls ramp the
            # HAM clock so real matmuls start at full speed.
            zt = wpool.tile([H, TILE], mybir.dt.bfloat16)
            nc.vector.memset(zt[:], 0.0)
            # dummy tanh: forces the ACT table load during the DMA head
            ztanh = wpool.tile([H, 1], f32)
            nc.scalar.activation(ztanh[:], cconst[:, 1:2], Tanh)
            for wi in range(NWARM):
                if VARIANT in ("A", "C"):
                    wps = pspool.tile([H, SPAN], f32, tag="l1", bufs=2)
                elif VARIANT == "E":
                    wps = pspool.tile([H, SPAN], f32, tag="l2", bufs=3)
                else:
                    wps = pspool.tile([M3, TILE], f32, tag="l3", bufs=2)
                nc.tensor.matmul(wps[0:16, 0:TILE], zt[:, 0:16], zt[:])

            pair_ps = None
            for s, sp in units:
                off, W = spans[sp]
                tiles = _tiles_for(W)
                col0 = s * C + off
                slot = s * len(spans) + sp

                xt = xpool.tile([D + 1, SPAN], f32r, tag="x")
                nc.sync.dma_start(xt[:, :W], xin[:, col0:col0 + W])

                def do_l1(net):
                    l1ps = pspool.tile(
                        [H, SPAN], f32, tag="l1",
                        bufs=1 if VARIANT in ("B", "E") else 2,
                        name=f"l1ps{net}")
                    lw = l1w_sb[:, (2 * s + net) * H:(2 * s + net + 1) * H]
                    for (to, tw) in tiles:
                        nc.tensor.matmul(l1ps[:, to:to + tw], lw,
                                         xt[:, to:to + tw])
                    return l1ps

                def do_h1(net, l1ps):
                    h1 = hpool.tile([H, SPAN], f32r, tag="h1", name=f"h1_{net}")
                    if (s, sp, net) in dve_h1:
                        wt = hpool.tile([H, SPAN], f32, tag="wt")
                        nc.vector._custom_dve(
                            P1, out=wt[:, :W], in0=l1ps[:, :W],
                            in1=cconst[:, 0:1], s0=-1.0, s1=B2, imm2=B1)
                        nc.vector._custom_dve(
                            P2, out=h1[:, :W], in0=wt[:, :W],
                            in1=cconst[:, 1:2], s0=E0, s1=E2, imm2=E1)
                    else:
                        nc.scalar.activation(h1[:, :W], l1ps[:, :W],
                                             Tanh, scale=float(A_SCALE))
                    return h1

                def do_l2(net, h1):
                    if VARIANT == "E":
                        l2ps = pspool.tile([H, SPAN], f32, tag="l2",
                                           bufs=3, name=f"l2ps{net}")
                    elif VARIANT == "A":
                        l2ps = pspool.tile([H, SPAN], f32, tag="l2",
                                           bufs=2, name=f"l2ps{net}")
                    elif VARIANT == "C":
                        l2ps = []
                        lw = l2w_sb[:, (2 * s + net) * H:
                                    (2 * s + net + 1) * H]
                        for (to, tw) in tiles:
                            ps = pspool.tile([H, TILE], f32, tag="l2",
                                             bufs=3, name=f"l2ps{net}_{to}")
                            nc.tensor.matmul(ps[:, :tw], lw,
                                             h1[:, to:to + tw])
                            l2ps.append(ps)
                        return l2ps
                    else:
                        l2ps = pspool.tile([H, SPAN], f32,
                                           tag="l2a" if net == 0 else "l2c",
                                           name=f"l2ps{net}")
                    lw = l2w_sb[:, (2 * s + net) * H:(2 * s + net + 1) * H]
                    for (to, tw) in tiles:
                        nc.tensor.matmul(l2ps[:, to:to + tw], lw,
                                         h1[:, to:to + tw])
                    return l2ps

                def do_h2(net, l2ps):
                    h2 = hpool.tile([H, SPAN], f32r, tag="h2", name=f"h2_{net}")
                    if VARIANT == "C":
                        for ps, (to, tw) in zip(l2ps, tiles):
                            nc.scalar.activation(
                                h2[:, to:to + tw], ps[:, :tw], Tanh,
                                bias=l2b_sb[:, 2 * s + net:2 * s + net + 1])
                    else:
                        nc.scalar.activation(
                            h2[:, :W], l2ps[:, :W], Tanh,
                            bias=l2b_sb[:, 2 * s + net:2 * s + net + 1])
                    return h2

                h2s = []
                if VARIANT == "C":
                    psa = do_l1(0)
                    psc = do_l1(1)
                    h1a = do_h1(0, psa)
                    h1c = do_h1(1, psc)
                    l2a = do_l2(0, h1a)
                    l2c = do_l2(1, h1c)
                    h2s = [do_h2(0, l2a), do_h2(1, l2c)]
                    if pair_ps is None:
                        pair_ps = pspool.tile([M6, TILE], f32, tag="l3",
                                              bufs=1)
                        pair_first = True
                    else:
                        pair_first = False
                    l3tgt = pair_ps[:]
                elif VARIANT == "E":
                    psa = do_l1(0)
                    psc = do_l1(1)
                    h1a = do_h1(0, psa)
                    h1c = do_h1(1, psc)
                    l2a = do_l2(0, h1a)
                    l2c = do_l2(1, h1c)
                    h2s = [do_h2(0, l2a), do_h2(1, l2c)]
                    l3tgt = l2a[0:M3, 0:TILE]
                else:
                    l2s = []
                    for net in range(2):
                        ps = do_l1(net)
                        h1 = do_h1(net, ps)
                        l2ps = do_l2(net, h1)
                        h2s.append(do_h2(net, l2ps))
                        l2s.append(l2ps)
                    l3full = pspool.tile([M3, TILE], f32, tag="l3", bufs=2)
                    l3tgt = l3full[:]
                n_mm = 2 * len(tiles)
                mi = 0
                for net in range(2):
                    for j, (to, tw) in enumerate(tiles):
                        blk = ((s * 2 + net) * KT + j) * M6
                        nc.tensor.matmul(
                            l3tgt[:, 0:tw],
                            l3w_sb[:, blk:blk + M6],
                            h2s[net][:, to:to + tw],
                            start=(pair_first and mi == 0),
                            stop=(not pair_first and mi == n_mm - 1),
                            skip_group_check=True,
                        )
                        mi += 1
                if not pair_first:
                    osb = opool.tile([M6, TILE], f32, tag="o")
                    tw0 = tiles[0][1]
                    nc.vector.tensor_copy(osb[:, :tw0], l3tgt[:, 0:tw0])
                    nc.sync.dma_start(out[:, sp, :tw0], osb[:, :tw0])
                    pair_ps = None

    nc.compile()
    return nc


def _get_kernel(C):
    if C not in _compiled:
        _compiled[C] = _build(C)
    return _compiled[C]


def kernel(obs, skill_ids, Wa1, ba1, Wa2, ba2, Wa3, ba3,
           Wc1, bc1, Wc2, bc2, Wc3, bc3):
    from concourse.bass_utils import run_bass_kernel_spmd

    obs = np.asarray(obs, dtype=np.float32)
    sids = np.asarray(skill_ids).astype(np.int64)
    Wa1, ba1, Wa2, ba2, Wa3, ba3 = [np.asarray(a, np.float32)
                                    for a in (Wa1, ba1, Wa2, ba2, Wa3, ba3)]
    Wc1, bc1, Wc2, bc2, Wc3, bc3 = [np.asarray(a, np.float32)
                                    for a in (Wc1, bc1, Wc2, bc2, Wc3, bc3)]

    counts = np.bincount(sids, minlength=S)
    order = np.argsort(sids, kind="stable")
    starts = np.zeros(S + 1, np.int64)
    starts[1:] = np.cumsum(counts)

    C = max(2176, int(-(-counts.max() // 128) * 128))
    nc = _get_kernel(C)
    spans = _spans_for(C)

    inv_a = np.float32(1.0 / A_SCALE)
    obsT = np.ascontiguousarray(obs.T)  # [D, B]

    in_maps = []
    for c in range(NCORES):
        xin = np.zeros((D + 1, 2 * C), np.float32)
        l1w = np.zeros((D + 1, 4 * H), np.float32)
        l2w = np.zeros((H, 4 * H), np.float32)
        l2b = np.zeros((H, 4), np.float32)
        l3w = np.zeros((H, 2 * 2 * KT * M6), np.float32)
        for sloc in range(2):
            skill = 2 * c + sloc
            cnt = int(counts[skill])
            toks = order[starts[skill]:starts[skill] + cnt]
            xin[:D, sloc * C:sloc * C + cnt] = obsT[:, toks]
            xin[D, sloc * C:sloc * C + cnt] = 1.0
            for net, (W1, b1, W2, b2) in enumerate(
                ((Wa1, ba1, Wa2, ba2), (Wc1, bc1, Wc2, bc2))
            ):
                blk = slice((2 * sloc + net) * H, (2 * sloc + net + 1) * H)
                l1w[:D, blk] = W1[skill] * inv_a
                l1w[D, blk] = b1[skill] * inv_a
                l2w[:, blk] = W2[skill]
                l2b[:, 2 * sloc + net] = b2[skill]
            for j in range(KT):
                ro = (sloc * KT + j) * G
                ab = ((sloc * 2 + 0) * KT + j) * M6 + ro
                cb = ((sloc * 2 + 1) * KT + j) * M6 + ro
                l3w[:, ab:ab + A] = Wa3[skill]
                l3w[:, cb + A] = Wc3[skill, :, 0]
        in_maps.append({"xin": xin, "l1w": l1w, "l2w": l2w,
                        "l2b": l2b, "l3w": l3w})

    res = run_bass_kernel_spmd(nc, in_maps, core_ids=list(range(NCORES)))

    logits = np.empty((B, A), np.float32)
    value = np.empty((B,), np.float32)
    for c in range(NCORES):
        ot = res.results[c]["out"]  # [M6, len(spans), TILE]
        per = np.empty((2 * C, G), np.float32)
        for sloc in range(2):
            for sp, (off, W) in enumerate(spans):
                for j, (to, tw) in enumerate(_tiles_for(W)):
                    r0 = (sloc * KT + j) * G
                    blk = ot[r0:r0 + G, sp, :tw]  # [G, tw]
                    t0 = sloc * C + off + to
                    per[t0:t0 + tw] = blk.T
        for sloc in range(2):
            skill = 2 * c + sloc
            cnt = int(counts[skill])
            if cnt == 0:
                continue
            toks = order[starts[skill]:starts[skill] + cnt]
            blk = per[sloc * C:sloc * C + cnt]
            logits[toks] = blk[:, 0:A] + ba3[skill]
            value[toks] = blk[:, A] + bc3[skill, 0]
    return logits, value
